# revision 10
# baseline (speedup 1.0000x reference)
"""Trainium2 Bass kernel for nn_Annotator (embedding -> LSTM over S*W -> word
max-pool -> small temporal-conv predictor -> log_softmax).

Strategy: data-parallel over batch. Each of the 8 NeuronCores runs the full
model on a 32-element batch shard with zero inter-core communication; the
host shards inputs and concatenates outputs.

Per-core pipeline:
  - dma_gather(transpose=True) fetches bf16 embedding rows straight into the
    transposed [E-partition, E-chunk, token] layout the matmul needs.
  - Bulk X-projection (x @ Wih.T) per 16-step window, N=512 matmuls; the LSTM
    bias is folded in during the PSUM->SBUF copy.
  - 512-step LSTM recurrence with Whh stationary (bf16, quad-interleaved gate
    layout so each 128-row hidden chunk's i/f/g/o tiles are adjacent); cell
    state in fp32, h in bf16.
  - Word max-pool accumulated on the fly; predictor temporal convolutions as
    shifted-slice PSUM accumulations; log_softmax over C via TensorE
    transpose + free-dim reductions.
"""

import os
import sys

for _p in ("/opt/trn_rl_repo", "/root/.axon_site/_ro/trn_rl_repo"):
    if os.path.isdir(_p) and _p not in sys.path:
        sys.path.insert(0, _p)

import ml_dtypes
import numpy as np

import concourse.bacc as bacc
import concourse.bass as bass
import concourse.mybir as mybir
from concourse import library_config
from concourse.bass_utils import run_bass_kernel_spmd
from concourse.tile import TileContext

AF = mybir.ActivationFunctionType
ALU = mybir.AluOpType
AX = mybir.AxisListType
F32 = mybir.dt.float32
BF16 = mybir.dt.bfloat16
I16 = mybir.dt.int16
PSUM = bass.MemorySpace.PSUM

# Problem dims (hardcoded per spec).
B, S = 256, 16
W = int(os.environ.get("KERNEL_W", "32"))  # words per sentence (32 in spec)
V, E, H, C = 30000, 256, 512, 5
D1, D2 = 2, 3
NCORES = 8
BL = B // NCORES            # batch per core = 32
T = S * W                   # total timesteps
G = 4 * H                   # gate rows = 2048
NQ = H // 128               # hidden quads = 4
KC = H // 128               # K chunks over H = 4
EC = E // 128               # K chunks over E = 2
WIN = 16                    # recurrence steps per X-projection window
NWIN = T // WIN
TOKW = WIN * BL             # tokens gathered per window = 512
NMT = G // 128              # gate m-tiles = 16


def _perm():
    """Permuted gate-row order: quad q -> [i_q, f_q, g_q, o_q] (128 rows each).

    Torch gate layout is [i(H); f(H); g(H); o(H)]. We reorder rows so that the
    four gate slices of one 128-row hidden chunk are adjacent, letting the
    elementwise phase run on contiguous [128, 4, BL] tiles per quad.
    """
    p = np.empty(G, np.int64)
    r = 0
    for q in range(NQ):
        for gt in (0, 1, 3, 2):  # i, f, o, g  (all sigmoids adjacent)
            base = gt * H + q * 128
            p[r:r + 128] = np.arange(base, base + 128)
            r += 128
    return p


def _step_mm_order():
    """TE matmul order within one recurrence step.

    (q, mt, k): quad q, m-tile mt (gate tile 4q+mt), K chunk k.
    k=0 for everything first (consumes h chunk 0, produced earliest by the
    previous step), then quad 0 finishes all its K chunks (so its elementwise
    chain - which produces h chunk 0 for the NEXT step - starts early), then
    the remaining quads.
    """
    order = []
    for q in range(NQ):
        for mt in range(4):
            order.append((q, mt, 0))
    for k in range(1, KC):
        for q in (0, 1):
            for mt in range(4):
                order.append((q, mt, k))
    for k in range(1, KC):
        for q in (2, 3):
            for mt in range(4):
                order.append((q, mt, k))
    return order


def build_program():
    nc = bacc.Bacc("TRN2")

    idx_d = nc.declare_dram_parameter("idx", [128, T * BL // 16], I16, isOutput=False)
    emb_d = nc.declare_dram_parameter("emb", [V, E], BF16, isOutput=False)
    whh_d = nc.declare_dram_parameter("whh_t", [128, KC, G], BF16, isOutput=False)
    wih_d = nc.declare_dram_parameter("wih_t", [128, EC, G], BF16, isOutput=False)
    bias_d = nc.declare_dram_parameter("bias", [128, NMT], F32, isOutput=False)
    w1_d = nc.declare_dram_parameter("w1", [128, KC, D1 + 1, C], BF16, isOutput=False)
    w2_d = nc.declare_dram_parameter("w2", [C, D2 + 1, C], F32, isOutput=False)
    b1_d = nc.declare_dram_parameter("b1", [C, 1], F32, isOutput=False)
    b2_d = nc.declare_dram_parameter("b2", [C, 1], F32, isOutput=False)
    id_d = nc.declare_dram_parameter("ident", [128, 128], F32, isOutput=False)
    out_d = nc.declare_dram_parameter("out", [S * BL, C], F32, isOutput=True)
    DEBUG = os.environ.get("KERNEL_DEBUG") is not None
    if DEBUG:
        dbg_xt = nc.declare_dram_parameter("dbg_xt", [128, EC, TOKW], BF16, isOutput=True)
        dbg_X = nc.declare_dram_parameter("dbg_X", [128, WIN, NMT, BL], F32, isOutput=True)
        dbg_h = nc.declare_dram_parameter("dbg_h", [128, KC, BL], BF16, isOutput=True)
        dbg_c = nc.declare_dram_parameter("dbg_c", [128, NQ, BL], F32, isOutput=True)
        dbg_reps = nc.declare_dram_parameter("dbg_reps", [128, KC, S, BL], BF16, isOutput=True)
        dbg_z = nc.declare_dram_parameter("dbg_z", [C, S * BL], F32, isOutput=True)

    mm_order = _step_mm_order()

    with TileContext(nc) as tc:
        nc.gpsimd.load_library(library_config.mlp)
        with (
            tc.tile_pool(name="singles", bufs=1) as sp,
            tc.tile_pool(name="xTp", bufs=3) as xTp,
            tc.tile_pool(name="Xp", bufs=3) as Xp,
            tc.tile_pool(name="sigp", bufs=2) as sigp,
            tc.tile_pool(name="histp", bufs=2) as histp,
            tc.tile_pool(name="tmpp", bufs=2) as tmpp,
        ):
            whh_s = sp.tile([128, KC, G], BF16)
            wih_s = sp.tile([128, EC, G], BF16)
            bias_s = sp.tile([128, NMT], F32)
            w1_s = sp.tile([128, KC, D1 + 1, C], BF16)
            w2_s = sp.tile([C, D2 + 1, C], F32)
            b1_s = sp.tile([C, 1], F32)
            b2_s = sp.tile([C, 1], F32)
            id_s = sp.tile([128, 128], F32)
            idx_s = sp.tile([128, T * BL // 16], I16)
            h0_s = sp.tile([128, KC, BL], BF16)   # zeros (t=0 input)
            c_s = sp.tile([128, NQ, BL], F32)
            reps = sp.tile([128, KC, S, BL], BF16)
            A_s = sp.tile([C, S * BL], F32)
            FA_s = sp.tile([C, D2 * BL], F32)
            z_s = sp.tile([C, S * BL], F32)

            nc.sync.dma_start(out=whh_s[:], in_=whh_d[:])
            nc.sync.dma_start(out=wih_s[:], in_=wih_d[:])
            nc.sync.dma_start(out=bias_s[:], in_=bias_d[:])
            nc.sync.dma_start(out=w1_s[:], in_=w1_d[:])
            nc.sync.dma_start(out=w2_s[:], in_=w2_d[:])
            nc.sync.dma_start(out=b1_s[:], in_=b1_d[:])
            nc.sync.dma_start(out=b2_s[:], in_=b2_d[:])
            nc.sync.dma_start(out=id_s[:], in_=id_d[:])
            nc.sync.dma_start(out=idx_s[:], in_=idx_d[:])
            nc.vector.memset(h0_s[:], 0.0)
            nc.vector.memset(c_s[:], 0.0)

            with (
                tc.tile_pool(name="psg", bufs=2, space=PSUM) as psgp,
                tc.tile_pool(name="psx", bufs=4, space=PSUM) as psxp,
            ):
                def emit_window(w):
                    xT = xTp.tile([128, EC, TOKW], BF16, tag="xT")
                    nc.gpsimd.dma_gather(
                        xT[:], emb_d[:], idx_s[:, 32 * w:32 * (w + 1)],
                        TOKW, TOKW, E, transpose=True,
                    )
                    if DEBUG and w == 0:
                        nc.sync.dma_start(out=dbg_xt[:], in_=xT[:])
                    Xw = Xp.tile([128, WIN, NQ, 4, BL], F32, tag="X")
                    for m in range(NMT):
                        xps = psxp.tile([128, TOKW], F32, tag="xps")
                        for j in range(EC):
                            nc.tensor.matmul(
                                xps[:],
                                wih_s[:, j, 128 * m:128 * (m + 1)],
                                xT[:, j, :],
                                start=(j == 0), stop=(j == EC - 1),
                            )
                        # PSUM -> SBUF with the LSTM bias for this gate tile
                        # folded in (out = in + bias[p]); alternate engines to
                        # balance ACT/DVE load.
                        nc.scalar.activation(
                            Xw[:, :, m // 4, m % 4, :], xps[:], AF.Identity,
                            bias=bias_s[:, m:m + 1],
                        )
                    return Xw

                state = {"hist": None}

                def emit_step(t, Xw):
                    tl = t % WIN
                    s_idx, w_idx = divmod(t, W)
                    if w_idx == 0:
                        state["prev_hist"] = state["hist"]
                        state["hist"] = histp.tile([128, NQ, BL, W], BF16,
                                                   tag="hist", name="hist")
                    hist = state["hist"]
                    if t == 0:
                        h_rhs = h0_s
                    elif w_idx == 0:
                        h_rhs = state["prev_hist"][:, :, :, W - 1]
                    else:
                        h_rhs = hist[:, :, :, w_idx - 1]
                    gps = psgp.tile([128, NQ, 4, BL], F32, tag="g", name="g")
                    nmm = 0
                    for (q, mt, k) in mm_order:
                        gtile = 4 * q + mt
                        nmm += 1
                        nc.tensor.matmul(
                            gps[:, q, mt, :],
                            whh_s[:, k, 128 * gtile:128 * (gtile + 1)],
                            h_rhs[:, k, :],
                            start=(nmm == 1), stop=(nmm == len(mm_order)),
                            skip_group_check=True,
                        )
                    for hh in range(2):  # half = quads 2h, 2h+1
                        qs = slice(2 * hh, 2 * hh + 2)
                        g = gps[:, qs, :, :]
                        nc.vector.tensor_add(g, g, Xw[:, tl, qs, :, :])
                        sig = sigp.tile([128, 2, 3, BL], F32, tag=f"s{hh}",
                                        name=f"s{hh}")
                        tg = tmpp.tile([128, 2, BL], F32, tag=f"tg{hh}",
                                       name=f"tg{hh}")
                        nc.scalar.activation(sig[:], gps[:, qs, 0:3, :], AF.Sigmoid)
                        nc.scalar.activation(tg[:], gps[:, qs, 3, :], AF.Tanh)
                        t1 = tmpp.tile([128, 2, BL], F32, tag=f"t1{hh}",
                                       name=f"t1{hh}")
                        t2 = tmpp.tile([128, 2, BL], F32, tag=f"t2{hh}",
                                       name=f"t2{hh}")
                        nc.vector.tensor_mul(t1[:], sig[:, :, 0, :], tg[:])
                        nc.vector.tensor_mul(t2[:], sig[:, :, 1, :], c_s[:, qs, :])
                        nc.vector.tensor_add(c_s[:, qs, :], t1[:], t2[:])
                        tc2 = tmpp.tile([128, 2, BL], F32, tag=f"tc{hh}",
                                        name=f"tc{hh}")
                        nc.scalar.activation(tc2[:], c_s[:, qs, :], AF.Tanh)
                        nc.vector.tensor_mul(hist[:, qs, :, w_idx],
                                             sig[:, :, 2, :], tc2[:])
                    if w_idx == W - 1:
                        # max-pool over the sentence's word axis (innermost)
                        nc.vector.tensor_reduce(
                            reps[:, :, s_idx, :], hist[:],
                            axis=AX.X, op=ALU.max,
                        )

                xws = {0: emit_window(0)}
                if DEBUG:
                    nc.sync.dma_start(out=dbg_X[:], in_=xws[0][:])
                if NWIN > 1:
                    xws[1] = emit_window(1)
                for w in range(NWIN):
                    for tl in range(WIN):
                        emit_step(w * WIN + tl, xws[w])
                    if w + 2 < NWIN:
                        xws[w + 2] = emit_window(w + 2)
                        if DEBUG and w == 0 and tl == 0:
                            nc.sync.dma_start(out=dbg_h[:],
                                              in_=state["hist"][:, :, :, 0])
                            nc.sync.dma_start(out=dbg_c[:], in_=c_s[:])
                    del xws[w]
                if DEBUG:
                    nc.sync.dma_start(out=dbg_reps[:], in_=reps[:])

            # ---- predictor ----
            with tc.tile_pool(name="psp", bufs=1, space=PSUM) as pspp:
                # A_pre[s] = sum_k reps[s-k] @ W1[k] via shifted psum accumulation
                psA = pspp.tile([128, S * BL], F32, tag="pA")
                for k in range(D1 + 1):
                    for kc in range(KC):
                        nc.tensor.matmul(
                            psA[0:C, BL * k:S * BL],
                            w1_s[:, kc, k, :],
                            reps[:, kc, 0:S - k, :],
                            start=(k == 0 and kc == 0),
                            stop=(k == D1 and kc == KC - 1),
                        )
                nc.scalar.activation(A_s[:], psA[0:C, :], AF.Tanh, bias=b1_s[:])

                # fallback branch: F = tanh(reps[:D2] @ W1[0] + b1) @ W2[0] + b2
                psF = pspp.tile([128, D2 * BL], F32, tag="pF")
                for kc in range(KC):
                    nc.tensor.matmul(
                        psF[0:C, :],
                        w1_s[:, kc, 0, :],
                        reps[:, kc, 0:D2, :],
                        start=(kc == 0), stop=(kc == KC - 1),
                    )
                nc.scalar.activation(FA_s[:], psF[0:C, :], AF.Tanh, bias=b1_s[:])
                psF2 = pspp.tile([128, D2 * BL], F32, tag="pF2")
                nc.tensor.matmul(psF2[0:C, :], w2_s[:, 0, :], FA_s[:],
                                 start=True, stop=True)

                # main branch: M[i] = sum_t A[i+t] @ W2[t] (i = 0..S-D2-1)
                psM = pspp.tile([128, (S - D2) * BL], F32, tag="pM")
                for tt in range(D2 + 1):
                    nc.tensor.matmul(
                        psM[0:C, :],
                        w2_s[:, tt, :],
                        A_s[:, BL * tt:BL * (tt + S - D2)],
                        start=(tt == 0), stop=(tt == D2),
                    )

                # assemble pre-softmax logits [C, S*BL] (+ b2)
                nc.scalar.activation(z_s[:, 0:D2 * BL], psF2[0:C, :], AF.Identity,
                                     bias=b2_s[:])
                nc.scalar.activation(z_s[:, D2 * BL:], psM[0:C, :], AF.Identity,
                                     bias=b2_s[:])

                if DEBUG:
                    nc.sync.dma_start(out=dbg_z[:], in_=z_s[:])
                # log_softmax over C (partition dim): transpose 128-col chunks
                # to [128, C], then reduce along the free dim.
                for j in range(S * BL // 128):
                    zt = pspp.tile([128, C], F32, tag="zt")
                    nc.tensor.transpose(
                        zt[:, 0:C], z_s[:, 128 * j:128 * (j + 1)], id_s[0:C, 0:C]
                    )
                    nrmax = tmpp.tile([128, 1], F32, tag="nrm")
                    nc.vector.tensor_reduce(nrmax[:], zt[:, 0:C], axis=AX.X,
                                            op=ALU.max, negate=True)
                    ez = tmpp.tile([128, C], F32, tag="ez")
                    nc.scalar.activation(ez[:], zt[:, 0:C], AF.Exp, bias=nrmax[:])
                    rs = tmpp.tile([128, 1], F32, tag="rs")
                    nc.vector.reduce_sum(rs[:], ez[:], axis=AX.X)
                    ls = tmpp.tile([128, 1], F32, tag="ls")
                    nc.scalar.activation(ls[:], rs[:], AF.Ln)
                    off = tmpp.tile([128, 1], F32, tag="off")
                    nc.vector.tensor_sub(off[:], nrmax[:], ls[:])
                    oj = tmpp.tile([128, C], F32, tag="oj")
                    nc.vector.tensor_scalar_add(oj[:], zt[:, 0:C], off[:])
                    nc.sync.dma_start(out=out_d[128 * j:128 * (j + 1), :], in_=oj[:])

    nc.finalize()
    return nc


_PROG = None


def _program():
    global _PROG
    if _PROG is None:
        _PROG = build_program()
    return _PROG


def _pack_shared(inputs):
    perm = _perm()
    Wih = np.asarray(inputs["Wih"], np.float32)
    Whh = np.asarray(inputs["Whh"], np.float32)
    b = (np.asarray(inputs["b_ih"], np.float32)
         + np.asarray(inputs["b_hh"], np.float32))
    W1 = np.asarray(inputs["W1"], np.float32)
    W2 = np.asarray(inputs["W2"], np.float32)
    b1 = np.asarray(inputs["b1"], np.float32)
    b2 = np.asarray(inputs["b2"], np.float32)

    shared = {
        "emb": np.ascontiguousarray(
            np.asarray(inputs["emb_table"], np.float32).astype(ml_dtypes.bfloat16)
        ),
        "whh_t": np.ascontiguousarray(
            Whh[perm].T.reshape(KC, 128, G).transpose(1, 0, 2).astype(ml_dtypes.bfloat16)
        ),
        "wih_t": np.ascontiguousarray(
            Wih[perm].T.reshape(EC, 128, G).transpose(1, 0, 2).astype(ml_dtypes.bfloat16)
        ),
        "bias": np.ascontiguousarray(b[perm].reshape(NMT, 128).T),
        "w1": np.ascontiguousarray(
            W1.reshape(D1 + 1, KC, 128, C).transpose(2, 1, 0, 3)
        ).astype(ml_dtypes.bfloat16),
        "w2": np.ascontiguousarray(W2.transpose(1, 0, 2)),
        "b1": np.ascontiguousarray(b1.reshape(C, 1)),
        "b2": np.ascontiguousarray(b2.reshape(C, 1)),
        "ident": np.eye(128, dtype=np.float32),
    }
    return shared


def _pack_idx(abstracts_shard):
    # [BL, S, W] int32 -> time-major flat token list, wrapped 16-wide for
    # dma_gather's index layout (index i lives at [i % 16, i // 16]).
    tm = np.ascontiguousarray(np.transpose(abstracts_shard, (1, 2, 0)))  # [S, W, BL]
    flat = tm.reshape(-1)
    blk = flat.reshape(-1, 16).T.astype(np.int16)   # [16, T*BL//16]
    arr = np.tile(blk, (8, 1))                      # replicate per Q7 core group
    return np.ascontiguousarray(arr)


def make_in_maps(inputs):
    abstracts = np.asarray(inputs["abstracts"], np.int32)
    shared = _pack_shared(inputs)
    in_maps = []
    for c in range(NCORES):
        m = dict(shared)
        m["idx"] = _pack_idx(abstracts[c * BL:(c + 1) * BL])
        in_maps.append(m)
    return in_maps


def run(inputs, trace=False):
    nc = _program()
    in_maps = make_in_maps(inputs)
    res = run_bass_kernel_spmd(nc, in_maps, list(range(NCORES)), trace=trace)
    outs = [
        res.results[i]["out"].reshape(S, BL, C).transpose(1, 2, 0)
        for i in range(NCORES)
    ]
    out = np.concatenate(outs, axis=0)
    return np.ascontiguousarray(out, np.float32), res


def kernel(**inputs):
    out, _ = run(inputs, trace=False)
    return out


def profile(inputs):
    _, res = run(inputs, trace=True)
    return res.exec_time_ns


# revision 11
# speedup vs baseline: 1.2647x; 1.2647x over previous
"""Trainium2 Bass kernel for nn_Annotator (embedding -> LSTM over S*W -> word
max-pool -> small temporal-conv predictor -> log_softmax).

Strategy: data-parallel over batch. Each of the 8 NeuronCores runs the full
model on a 32-element batch shard with zero inter-core communication; the
host shards inputs and concatenates outputs.

Per-core pipeline:
  - dma_gather(transpose=True) fetches bf16 embedding rows straight into the
    transposed [E-partition, E-chunk, token] layout the matmul needs.
  - Bulk X-projection (x @ Wih.T) per 16-step window, N=512 matmuls; the LSTM
    bias is folded in during the PSUM->SBUF copy.
  - 512-step LSTM recurrence with Whh stationary (bf16, quad-interleaved gate
    layout so each 128-row hidden chunk's i/f/g/o tiles are adjacent); cell
    state in fp32, h in bf16.
  - Word max-pool accumulated on the fly; predictor temporal convolutions as
    shifted-slice PSUM accumulations; log_softmax over C via TensorE
    transpose + free-dim reductions.
"""

import os
import sys

for _p in ("/opt/trn_rl_repo", "/root/.axon_site/_ro/trn_rl_repo"):
    if os.path.isdir(_p) and _p not in sys.path:
        sys.path.insert(0, _p)

import ml_dtypes
import numpy as np

import concourse.bacc as bacc
import concourse.bass as bass
import concourse.mybir as mybir
from concourse import library_config
from concourse.bass_utils import run_bass_kernel_spmd
from concourse.tile import TileContext

AF = mybir.ActivationFunctionType
ALU = mybir.AluOpType
AX = mybir.AxisListType
F32 = mybir.dt.float32
BF16 = mybir.dt.bfloat16
I16 = mybir.dt.int16
PSUM = bass.MemorySpace.PSUM

# Problem dims (hardcoded per spec).
B, S = 256, 16
W = int(os.environ.get("KERNEL_W", "32"))  # words per sentence (32 in spec)
V, E, H, C = 30000, 256, 512, 5
D1, D2 = 2, 3
NCORES = 8
BL = B // NCORES            # batch per core = 32
T = S * W                   # total timesteps
G = 4 * H                   # gate rows = 2048
NQ = H // 128               # hidden quads = 4
KC = H // 128               # K chunks over H = 4
EC = E // 128               # K chunks over E = 2
WIN = 16                    # recurrence steps per X-projection window
NWIN = T // WIN
TOKW = WIN * BL             # tokens gathered per window = 512
NMT = G // 128              # gate m-tiles = 16


def _perm():
    """Permuted gate-row order: quad q -> [i_q, f_q, g_q, o_q] (128 rows each).

    Torch gate layout is [i(H); f(H); g(H); o(H)]. We reorder rows so that the
    four gate slices of one 128-row hidden chunk are adjacent, letting the
    elementwise phase run on contiguous [128, 4, BL] tiles per quad.
    """
    p = np.empty(G, np.int64)
    r = 0
    for q in range(NQ):
        for gt in (0, 1, 3, 2):  # i, f, o, g  (all sigmoids adjacent)
            base = gt * H + q * 128
            p[r:r + 128] = np.arange(base, base + 128)
            r += 128
    return p


def _step_mm_order():
    """TE matmul order within one recurrence step.

    (q, mt, k): quad q, m-tile mt (gate tile 4q+mt), K chunk k.
    k=0 for everything first (consumes h chunk 0, produced earliest by the
    previous step), then quad 0 finishes all its K chunks (so its elementwise
    chain - which produces h chunk 0 for the NEXT step - starts early), then
    the remaining quads.
    """
    order = []
    for q in range(NQ):
        for mt in range(4):
            order.append((q, mt, 0))
    for k in range(1, KC):
        for q in (0, 1):
            for mt in range(4):
                order.append((q, mt, k))
    for k in range(1, KC):
        for q in (2, 3):
            for mt in range(4):
                order.append((q, mt, k))
    return order


def build_program():
    nc = bacc.Bacc("TRN2")

    idx_d = nc.declare_dram_parameter("idx", [128, T * BL // 16], I16, isOutput=False)
    emb_d = nc.declare_dram_parameter("emb", [V, E], BF16, isOutput=False)
    whh_d = nc.declare_dram_parameter("whh_t", [128, KC, G], BF16, isOutput=False)
    wih_d = nc.declare_dram_parameter("wih_t", [128, EC, G], BF16, isOutput=False)
    bias_d = nc.declare_dram_parameter("bias", [128, NMT], F32, isOutput=False)
    w1_d = nc.declare_dram_parameter("w1", [128, KC, D1 + 1, C], BF16, isOutput=False)
    w2_d = nc.declare_dram_parameter("w2", [C, D2 + 1, C], F32, isOutput=False)
    b1_d = nc.declare_dram_parameter("b1", [C, 1], F32, isOutput=False)
    b2_d = nc.declare_dram_parameter("b2", [C, 1], F32, isOutput=False)
    id_d = nc.declare_dram_parameter("ident", [128, 128], F32, isOutput=False)
    out_d = nc.declare_dram_parameter("out", [S * BL, C], F32, isOutput=True)
    DEBUG = os.environ.get("KERNEL_DEBUG") is not None
    if DEBUG:
        dbg_xt = nc.declare_dram_parameter("dbg_xt", [128, EC, TOKW], BF16, isOutput=True)
        dbg_X = nc.declare_dram_parameter("dbg_X", [128, WIN, NMT, BL], F32, isOutput=True)
        dbg_h = nc.declare_dram_parameter("dbg_h", [128, KC, BL], BF16, isOutput=True)
        dbg_c = nc.declare_dram_parameter("dbg_c", [128, NQ, BL], F32, isOutput=True)
        dbg_reps = nc.declare_dram_parameter("dbg_reps", [128, KC, S, BL], BF16, isOutput=True)
        dbg_z = nc.declare_dram_parameter("dbg_z", [C, S * BL], F32, isOutput=True)

    mm_order = _step_mm_order()

    with TileContext(nc) as tc:
        nc.gpsimd.load_library(library_config.mlp)
        with (
            tc.tile_pool(name="singles", bufs=1) as sp,
            tc.tile_pool(name="xTp", bufs=3) as xTp,
            tc.tile_pool(name="Xp", bufs=3) as Xp,
            tc.tile_pool(name="sigp", bufs=2) as sigp,
            tc.tile_pool(name="histp", bufs=2) as histp,
            tc.tile_pool(name="tmpp", bufs=2) as tmpp,
        ):
            whh_s = sp.tile([128, KC, G], BF16)
            wih_s = sp.tile([128, EC, G], BF16)
            bias_s = sp.tile([128, NMT], F32)
            w1_s = sp.tile([128, KC, D1 + 1, C], BF16)
            w2_s = sp.tile([C, D2 + 1, C], F32)
            b1_s = sp.tile([C, 1], F32)
            b2_s = sp.tile([C, 1], F32)
            id_s = sp.tile([128, 128], F32)
            idx_s = sp.tile([128, T * BL // 16], I16)
            h0_s = sp.tile([128, KC, BL], BF16)   # zeros (t=0 input)
            c_s = sp.tile([128, NQ, BL], F32)
            reps = sp.tile([128, KC, S, BL], BF16)
            A_s = sp.tile([C, S * BL], F32)
            FA_s = sp.tile([C, D2 * BL], F32)
            z_s = sp.tile([C, S * BL], F32)

            nc.sync.dma_start(out=whh_s[:], in_=whh_d[:])
            nc.sync.dma_start(out=wih_s[:], in_=wih_d[:])
            nc.sync.dma_start(out=bias_s[:], in_=bias_d[:])
            nc.sync.dma_start(out=w1_s[:], in_=w1_d[:])
            nc.sync.dma_start(out=w2_s[:], in_=w2_d[:])
            nc.sync.dma_start(out=b1_s[:], in_=b1_d[:])
            nc.sync.dma_start(out=b2_s[:], in_=b2_d[:])
            nc.sync.dma_start(out=id_s[:], in_=id_d[:])
            nc.sync.dma_start(out=idx_s[:], in_=idx_d[:])
            nc.vector.memset(h0_s[:], 0.0)
            nc.vector.memset(c_s[:], 0.0)

            with (
                tc.tile_pool(name="psg", bufs=2, space=PSUM) as psgp,
                tc.tile_pool(name="psx", bufs=4, space=PSUM) as psxp,
            ):
                def emit_window(w):
                    xT = xTp.tile([128, EC, TOKW], BF16, tag="xT")
                    nc.gpsimd.dma_gather(
                        xT[:], emb_d[:], idx_s[:, 32 * w:32 * (w + 1)],
                        TOKW, TOKW, E, transpose=True,
                    )
                    if DEBUG and w == 0:
                        nc.sync.dma_start(out=dbg_xt[:], in_=xT[:])
                    Xw = Xp.tile([128, WIN, NQ, 4, BL], F32, tag="X")
                    for m in range(NMT):
                        xps = psxp.tile([128, TOKW], F32, tag="xps")
                        for j in range(EC):
                            nc.tensor.matmul(
                                xps[:],
                                wih_s[:, j, 128 * m:128 * (m + 1)],
                                xT[:, j, :],
                                start=(j == 0), stop=(j == EC - 1),
                            )
                        # PSUM -> SBUF with the LSTM bias for this gate tile
                        # folded in (out = in + bias[p]); alternate engines to
                        # balance ACT/DVE load.
                        nc.scalar.activation(
                            Xw[:, :, m // 4, m % 4, :], xps[:], AF.Identity,
                            bias=bias_s[:, m:m + 1],
                        )
                    return Xw

                state = {"hist": None}

                def emit_step(t, Xw):
                    tl = t % WIN
                    s_idx, w_idx = divmod(t, W)
                    if w_idx == 0:
                        state["prev_hist"] = state["hist"]
                        state["hist"] = histp.tile([128, NQ, W, BL], BF16,
                                                   tag="hist", name="hist")
                    hist = state["hist"]
                    if t == 0:
                        h_rhs = h0_s
                    elif w_idx == 0:
                        h_rhs = state["prev_hist"][:, :, W - 1, :]
                    else:
                        h_rhs = hist[:, :, w_idx - 1, :]
                    gps = psgp.tile([128, NQ, 4, BL], F32, tag="g", name="g")
                    nmm = 0
                    for (q, mt, k) in mm_order:
                        gtile = 4 * q + mt
                        nmm += 1
                        nc.tensor.matmul(
                            gps[:, q, mt, :],
                            whh_s[:, k, 128 * gtile:128 * (gtile + 1)],
                            h_rhs[:, k, :],
                            start=(nmm == 1), stop=(nmm == len(mm_order)),
                            skip_group_check=True,
                        )
                    for hh in range(2):  # half = quads 2h, 2h+1
                        qs = slice(2 * hh, 2 * hh + 2)
                        g = gps[:, qs, :, :]
                        nc.vector.tensor_add(g, g, Xw[:, tl, qs, :, :])
                        sig = sigp.tile([128, 2, 3, BL], F32, tag=f"s{hh}",
                                        name=f"s{hh}")
                        tg = tmpp.tile([128, 2, BL], F32, tag=f"tg{hh}",
                                       name=f"tg{hh}")
                        nc.scalar.activation(sig[:], gps[:, qs, 0:3, :], AF.Sigmoid)
                        nc.scalar.activation(tg[:], gps[:, qs, 3, :], AF.Tanh)
                        t1 = tmpp.tile([128, 2, BL], F32, tag=f"t1{hh}",
                                       name=f"t1{hh}")
                        t2 = tmpp.tile([128, 2, BL], F32, tag=f"t2{hh}",
                                       name=f"t2{hh}")
                        nc.vector.tensor_mul(t1[:], sig[:, :, 0, :], tg[:])
                        nc.vector.tensor_mul(t2[:], sig[:, :, 1, :], c_s[:, qs, :])
                        nc.vector.tensor_add(c_s[:, qs, :], t1[:], t2[:])
                        tc2 = tmpp.tile([128, 2, BL], F32, tag=f"tc{hh}",
                                        name=f"tc{hh}")
                        nc.scalar.activation(tc2[:], c_s[:, qs, :], AF.Tanh)
                        nc.vector.tensor_mul(hist[:, qs, w_idx, :],
                                             sig[:, :, 2, :], tc2[:])
                    if w_idx == W - 1:
                        # max-pool over the word axis: halving tree on DVE
                        pw = W
                        src_ap = hist
                        while pw > 2:
                            half = pw // 2
                            nc.vector.tensor_max(
                                src_ap[:, :, 0:half, :],
                                src_ap[:, :, 0:half, :],
                                src_ap[:, :, half:pw, :],
                            )
                            pw = half
                        nc.vector.tensor_max(
                            reps[:, :, s_idx, :],
                            src_ap[:, :, 0, :], src_ap[:, :, 1, :],
                        )

                xws = {0: emit_window(0)}
                if DEBUG:
                    nc.sync.dma_start(out=dbg_X[:], in_=xws[0][:])
                if NWIN > 1:
                    xws[1] = emit_window(1)
                for w in range(NWIN):
                    for tl in range(WIN):
                        emit_step(w * WIN + tl, xws[w])
                    if w + 2 < NWIN:
                        xws[w + 2] = emit_window(w + 2)
                        if DEBUG and w == 0 and tl == 0:
                            nc.sync.dma_start(out=dbg_h[:],
                                              in_=state["hist"][:, :, 0, :])
                            nc.sync.dma_start(out=dbg_c[:], in_=c_s[:])
                    del xws[w]
                if DEBUG:
                    nc.sync.dma_start(out=dbg_reps[:], in_=reps[:])

            # ---- predictor ----
            with tc.tile_pool(name="psp", bufs=1, space=PSUM) as pspp:
                # A_pre[s] = sum_k reps[s-k] @ W1[k] via shifted psum accumulation
                psA = pspp.tile([128, S * BL], F32, tag="pA")
                for k in range(D1 + 1):
                    for kc in range(KC):
                        nc.tensor.matmul(
                            psA[0:C, BL * k:S * BL],
                            w1_s[:, kc, k, :],
                            reps[:, kc, 0:S - k, :],
                            start=(k == 0 and kc == 0),
                            stop=(k == D1 and kc == KC - 1),
                        )
                nc.scalar.activation(A_s[:], psA[0:C, :], AF.Tanh, bias=b1_s[:])

                # fallback branch: F = tanh(reps[:D2] @ W1[0] + b1) @ W2[0] + b2
                psF = pspp.tile([128, D2 * BL], F32, tag="pF")
                for kc in range(KC):
                    nc.tensor.matmul(
                        psF[0:C, :],
                        w1_s[:, kc, 0, :],
                        reps[:, kc, 0:D2, :],
                        start=(kc == 0), stop=(kc == KC - 1),
                    )
                nc.scalar.activation(FA_s[:], psF[0:C, :], AF.Tanh, bias=b1_s[:])
                psF2 = pspp.tile([128, D2 * BL], F32, tag="pF2")
                nc.tensor.matmul(psF2[0:C, :], w2_s[:, 0, :], FA_s[:],
                                 start=True, stop=True)

                # main branch: M[i] = sum_t A[i+t] @ W2[t] (i = 0..S-D2-1)
                psM = pspp.tile([128, (S - D2) * BL], F32, tag="pM")
                for tt in range(D2 + 1):
                    nc.tensor.matmul(
                        psM[0:C, :],
                        w2_s[:, tt, :],
                        A_s[:, BL * tt:BL * (tt + S - D2)],
                        start=(tt == 0), stop=(tt == D2),
                    )

                # assemble pre-softmax logits [C, S*BL] (+ b2)
                nc.scalar.activation(z_s[:, 0:D2 * BL], psF2[0:C, :], AF.Identity,
                                     bias=b2_s[:])
                nc.scalar.activation(z_s[:, D2 * BL:], psM[0:C, :], AF.Identity,
                                     bias=b2_s[:])

                if DEBUG:
                    nc.sync.dma_start(out=dbg_z[:], in_=z_s[:])
                # log_softmax over C (partition dim): transpose 128-col chunks
                # to [128, C], then reduce along the free dim.
                for j in range(S * BL // 128):
                    zt = pspp.tile([128, C], F32, tag="zt")
                    nc.tensor.transpose(
                        zt[:, 0:C], z_s[:, 128 * j:128 * (j + 1)], id_s[0:C, 0:C]
                    )
                    nrmax = tmpp.tile([128, 1], F32, tag="nrm")
                    nc.vector.tensor_reduce(nrmax[:], zt[:, 0:C], axis=AX.X,
                                            op=ALU.max, negate=True)
                    ez = tmpp.tile([128, C], F32, tag="ez")
                    nc.scalar.activation(ez[:], zt[:, 0:C], AF.Exp, bias=nrmax[:])
                    rs = tmpp.tile([128, 1], F32, tag="rs")
                    nc.vector.reduce_sum(rs[:], ez[:], axis=AX.X)
                    ls = tmpp.tile([128, 1], F32, tag="ls")
                    nc.scalar.activation(ls[:], rs[:], AF.Ln)
                    off = tmpp.tile([128, 1], F32, tag="off")
                    nc.vector.tensor_sub(off[:], nrmax[:], ls[:])
                    oj = tmpp.tile([128, C], F32, tag="oj")
                    nc.vector.tensor_scalar_add(oj[:], zt[:, 0:C], off[:])
                    nc.sync.dma_start(out=out_d[128 * j:128 * (j + 1), :], in_=oj[:])

    nc.finalize()
    return nc


_PROG = None


def _program():
    global _PROG
    if _PROG is None:
        _PROG = build_program()
    return _PROG


def _pack_shared(inputs):
    perm = _perm()
    Wih = np.asarray(inputs["Wih"], np.float32)
    Whh = np.asarray(inputs["Whh"], np.float32)
    b = (np.asarray(inputs["b_ih"], np.float32)
         + np.asarray(inputs["b_hh"], np.float32))
    W1 = np.asarray(inputs["W1"], np.float32)
    W2 = np.asarray(inputs["W2"], np.float32)
    b1 = np.asarray(inputs["b1"], np.float32)
    b2 = np.asarray(inputs["b2"], np.float32)

    shared = {
        "emb": np.ascontiguousarray(
            np.asarray(inputs["emb_table"], np.float32).astype(ml_dtypes.bfloat16)
        ),
        "whh_t": np.ascontiguousarray(
            Whh[perm].T.reshape(KC, 128, G).transpose(1, 0, 2).astype(ml_dtypes.bfloat16)
        ),
        "wih_t": np.ascontiguousarray(
            Wih[perm].T.reshape(EC, 128, G).transpose(1, 0, 2).astype(ml_dtypes.bfloat16)
        ),
        "bias": np.ascontiguousarray(b[perm].reshape(NMT, 128).T),
        "w1": np.ascontiguousarray(
            W1.reshape(D1 + 1, KC, 128, C).transpose(2, 1, 0, 3)
        ).astype(ml_dtypes.bfloat16),
        "w2": np.ascontiguousarray(W2.transpose(1, 0, 2)),
        "b1": np.ascontiguousarray(b1.reshape(C, 1)),
        "b2": np.ascontiguousarray(b2.reshape(C, 1)),
        "ident": np.eye(128, dtype=np.float32),
    }
    return shared


def _pack_idx(abstracts_shard):
    # [BL, S, W] int32 -> time-major flat token list, wrapped 16-wide for
    # dma_gather's index layout (index i lives at [i % 16, i // 16]).
    tm = np.ascontiguousarray(np.transpose(abstracts_shard, (1, 2, 0)))  # [S, W, BL]
    flat = tm.reshape(-1)
    blk = flat.reshape(-1, 16).T.astype(np.int16)   # [16, T*BL//16]
    arr = np.tile(blk, (8, 1))                      # replicate per Q7 core group
    return np.ascontiguousarray(arr)


def make_in_maps(inputs):
    abstracts = np.asarray(inputs["abstracts"], np.int32)
    shared = _pack_shared(inputs)
    in_maps = []
    for c in range(NCORES):
        m = dict(shared)
        m["idx"] = _pack_idx(abstracts[c * BL:(c + 1) * BL])
        in_maps.append(m)
    return in_maps


def run(inputs, trace=False):
    nc = _program()
    in_maps = make_in_maps(inputs)
    res = run_bass_kernel_spmd(nc, in_maps, list(range(NCORES)), trace=trace)
    outs = [
        res.results[i]["out"].reshape(S, BL, C).transpose(1, 2, 0)
        for i in range(NCORES)
    ]
    out = np.concatenate(outs, axis=0)
    return np.ascontiguousarray(out, np.float32), res


def kernel(**inputs):
    out, _ = run(inputs, trace=False)
    return out


def profile(inputs):
    _, res = run(inputs, trace=True)
    return res.exec_time_ns
